# revision 1
# baseline (speedup 1.0000x reference)
"""Trainium2 Bass kernel for nn_CrossAttention2 (8 cores, data-parallel over batch).

Reference computation (per batch element b):
    q = Wq @ x_b + bq          # [512, 1024]   x_b = x[b].reshape(512, 32*32)
    k = Wk @ c_b + bk          # c_b = context[b]
    v = Wv @ c_b + bv
    per head h (8 heads x 64 dim):
        S_h = (Q_h^T @ K_h) / 8        # [1024q, 1024k]
        A_h = softmax(S_h, axis=k)
        out_h = V_h @ A_h              # contraction over the QUERY axis (faithful
                                       # to the original module's quirk)
    y_b = Wo @ concat(out_h) + bo

Sharding: one batch element per NeuronCore (BS == n_cores == 8), no collectives.

All inputs are packed host-side into ONE [128, BLOB_F] float32 blob per core so
the kernel issues a single input DMA (DMA instructions have high fixed cost on
this runtime) and a single output DMA.

Blob layout (free-dim element offsets, per partition p):
    [    0,  4096)  xT      x[kc*128+p, hw]       -> 4 chunks * 1024
    [ 4096,  8192)  ctxT
    [ 8192, 10240)  wqT     Wq.T/8 [kc*128+p, i]  -> 4 chunks * 512
    [10240, 12288)  wkT
    [12288, 14336)  wvT
    [14336, 16384)  woT
    [16384, 16388)  bo      [p, m] column form (C = m*128+p) for the ACT bias
    [16388, 16900)  bq/8    row on partition 0 (rank-1 bias matmul lhsT)
    [16900, 17412)  bk      row on partition 0
    [17412, 17924)  bv      row on partition 0
    [17924, 18436)  ones    row of 1.0 on partition 0
BLOB_F = 18464

Matmul dtype is float32r (1 row/cycle at N=512 on the PE vs 4 for fp32;
measured rel err ~6e-4 end to end). Set MM_DT = F32 for a full-precision
fallback (~1e-6) at ~4x the PE cycles.
Q/K/V biases are folded in as rank-1 PE updates (bias_row^T (x) ones_row), so
ScalarE only runs the 64 exp instructions plus the 8 output-projection
bias-adds. Softmax skips max-subtraction (scores are O(+-6); exp cannot
overflow; softmax is shift-invariant). Normalization is folded into V^T rows
(16x fewer elements than scaling probabilities); reciprocals are batched 4
query-tiles at a time. The AV accumulation of head h is emitted after the
scores/softmax of head h+1 so the PE never waits on the exp chain.
"""

import numpy as np
from contextlib import ExitStack

import concourse.bass as bass
from concourse import bacc
import concourse.tile as tile
from concourse import mybir
from concourse.bass_utils import run_bass_kernel_spmd

F32 = mybir.dt.float32
F32R = mybir.dt.float32r

BS, C, H, W = 8, 512, 32, 32
HW = H * W
N_HEADS, DIM_HEAD = 8, 64
INNER = N_HEADS * DIM_HEAD
N_CORES = 8

MM_DT = F32R

OFF_X = 0
OFF_CTX = 4096
OFF_WQ = 8192
OFF_WK = 10240
OFF_WV = 12288
OFF_WO = 14336
OFF_BO = 16384
OFF_BQR = 16388
OFF_BKR = 16900
OFF_BVR = 17412
OFF_ONES = 17924
BLOB_F = 18464


def _mm(nc, out, lhsT, rhs, start, stop):
    nc.tensor.matmul(out, lhsT, rhs, start=start, stop=stop)


def make_pools(ctx: ExitStack, tc: tile.TileContext):
    const = ctx.enter_context(tc.tile_pool(name="const", bufs=1))
    # PSUM: 8 banks of 2KB. sp: [128,1024] f32 = 2 banks x 3; av: 2 banks x 1.
    sp = ctx.enter_context(tc.tile_pool(name="sp", bufs=2, space="PSUM"))
    av = ctx.enter_context(tc.tile_pool(name="av", bufs=2, space="PSUM"))
    sm = ctx.enter_context(tc.tile_pool(name="sm", bufs=8))
    ppool = ctx.enter_context(tc.tile_pool(name="probs", bufs=8))
    vpool = ctx.enter_context(tc.tile_pool(name="vsc", bufs=3))
    return const, sp, av, sm, ppool, vpool


def _kernel_body(ctx: ExitStack, tc: tile.TileContext, io: dict, pools=None,
                 variant="full", blob=None, skip_out=False):
    nc = tc.nc
    if pools is None:
        pools = make_pools(ctx, tc)
    const, sp, av, sm, ppool, vpool = pools

    if blob is None:
        blob = const.tile([128, BLOB_F], MM_DT, tag="blob")
        nc.sync.dma_start(out=blob[:], in_=io["blob"])

    def seg(off, ln):
        return blob[:, off:off + ln]

    Xs = seg(OFF_X, 4096).rearrange("p (kc f) -> p kc f", kc=4)
    Cs = seg(OFF_CTX, 4096).rearrange("p (kc f) -> p kc f", kc=4)
    WqT = seg(OFF_WQ, 2048).rearrange("p (kc f) -> p kc f", kc=4)
    WkT = seg(OFF_WK, 2048).rearrange("p (kc f) -> p kc f", kc=4)
    WvT = seg(OFF_WV, 2048).rearrange("p (kc f) -> p kc f", kc=4)
    WoT = seg(OFF_WO, 2048).rearrange("p (kc f) -> p kc f", kc=4)
    bo = seg(OFF_BO, 4).bitcast(F32)
    bq_row = blob[0:1, OFF_BQR:OFF_BQR + 512]
    bk_row = blob[0:1, OFF_BKR:OFF_BKR + 512]
    bv_row = blob[0:1, OFF_BVR:OFF_BVR + 512]
    ones = blob[0:1, OFF_ONES:OFF_ONES + 512]

    Q = const.tile([128, 4, 1024], MM_DT, tag="Q")
    K = const.tile([128, 4, 1024], MM_DT, tag="K")
    Vt = const.tile([128, 8, 512], MM_DT, tag="Vt")
    O = const.tile([128, 4, 1024], MM_DT, tag="O")
    Y = const.tile([128, 4, 1024], F32, tag="Y")

    if variant == "dma":
        for m in range(4):
            nc.vector.tensor_copy(out=Y[:, m, :],
                                  in_=Cs[:, m, :].bitcast(F32))
        nc.sync.dma_start(out=io["y"], in_=Y[:])
        return

    # ---- projections + attention, stage-interleaved ----
    # Emission order sets Tile's priorities. Interleave Q/K chunk m with the
    # scores/exp of heads 2m,2m+1 so ScalarE's 64-exp stream (the bottleneck
    # engine) starts ~2.5us into the kernel instead of after all projections.
    def emit_qk(m):
        for dst, w, brow, src, nm in ((Q, WqT, bq_row, Xs, "q"),
                                      (K, WkT, bk_row, Cs, "k")):
            ps = sp.tile([128, 1024], F32, tag="sp", name=f"ps_p{nm}{m}")
            for n in range(2):
                for kc in range(4):
                    _mm(nc, ps[:, n * 512:(n + 1) * 512],
                        w[:, kc, m * 128:(m + 1) * 128],
                        src[:, kc, n * 512:(n + 1) * 512],
                        kc == 0, False)
                _mm(nc, ps[:, n * 512:(n + 1) * 512],
                    brow[:, m * 128:(m + 1) * 128], ones, False, True)
            nc.vector.tensor_copy(out=dst[:, m, :], in_=ps[:])

    def emit_v():
        for jt in range(8):
            ps = sp.tile([128, 1024], F32, tag="sp", name=f"ps_v{jt}")
            for kc in range(4):
                _mm(nc, ps[:, 0:512],
                    Cs[:, kc, jt * 128:(jt + 1) * 128],
                    WvT[:, kc, :],
                    kc == 0, False)
            _mm(nc, ps[:, 0:512], ones[:, 0:128], bv_row, False, True)
            nc.vector.tensor_copy(out=Vt[:, jt, :], in_=ps[:, 0:512])

    def emit_scores_exp(h):
        m_h, p0 = h // 2, (h % 2) * 64
        Qh = Q[p0:p0 + 64, m_h, :]
        Kh = K[p0:p0 + 64, m_h, :]
        probs_l = []
        sums = sm.tile([128, 8], F32, tag="sums", name=f"sums{h}")
        for qt in range(8):
            ps = sp.tile([128, 1024], F32, tag="sp", name=f"ps_s{h}_{qt}")
            qslice = Qh[:, qt * 128:(qt + 1) * 128]
            _mm(nc, ps[:, 0:512], qslice, Kh[:, 0:512], True, True)
            _mm(nc, ps[:, 512:1024], qslice, Kh[:, 512:1024], True, True)
            probs = ppool.tile([128, 1024], MM_DT, tag="probs",
                               name=f"probs{h}_{qt}")
            if variant == "noexp":
                nc.scalar.copy(probs[:], ps[:])
            else:
                nc.scalar.activation(out=probs[:], in_=ps[:],
                                     func=mybir.ActivationFunctionType.Exp,
                                     accum_out=sums[:, qt:qt + 1])
            probs_l.append(probs)
        return probs_l, sums

    def emit_recip_vsc(h, sums):
        if variant == "noexp":
            return None
        rec = sm.tile([128, 8], F32, tag="rec", name=f"rec{h}")
        nc.vector.reciprocal(out=rec[:], in_=sums[:])
        # vsc[p, qt, d] = Vt[p, qt, h*64+d] * rec[p, qt] (stride-0 broadcast)
        vsc_t = vpool.tile([128, 8, 64], MM_DT, tag="vsc", name=f"vsc{h}")
        rec_b = bass.AP(tensor=rec.tensor, offset=rec[:].offset,
                        ap=[rec[:].ap[0], rec[:].ap[1], [0, 64]])
        nc.vector.tensor_mul(vsc_t[:], Vt[:, :, h * 64:(h + 1) * 64], rec_b)
        return vsc_t

    def emit_av(h, probs_l, vsc_t):
        m_h, p0 = h // 2, (h % 2) * 64
        po = av.tile([64, 1024], F32, tag="av", name=f"po{h}")
        for qt in range(8):
            if variant == "noexp":
                vsc = Vt[:, qt, h * 64:(h + 1) * 64]
            else:
                vsc = vsc_t[:, qt, :]
            probs = probs_l[qt]
            _mm(nc, po[:, 0:512], vsc, probs[:, 0:512], qt == 0, qt == 7)
            _mm(nc, po[:, 512:1024], vsc, probs[:, 512:1024],
                qt == 0, qt == 7)
        nc.vector.tensor_copy(out=O[p0:p0 + 64, m_h, :], in_=po[:])

    if variant == "proj":
        for m in range(4):
            emit_qk(m)
        emit_v()
        for m in range(4):
            nc.vector.tensor_copy(
                out=O[:, m, :],
                in_=Vt[:, 2 * m:2 * m + 2, :].rearrange("p a b -> p (a b)"))
    else:
        # stage pipeline: QK(m) ahead of heads 2m/2m+1; V right after the
        # first scores; recip/vsc one stage later; AV two stages later.
        # one-head AV lag: probs demand stays at 2 heads (16 tiles) against
        # the 8-slot pool instead of 3 heads with a two-head lag.
        se = {}
        rv = {}
        emit_qk(0)
        se[0] = emit_scores_exp(0)
        emit_v()
        emit_qk(1)
        se[1] = emit_scores_exp(1)
        rv[0] = emit_recip_vsc(0, se[0][1])
        emit_av(0, se[0][0], rv[0])
        emit_qk(2)
        se[2] = emit_scores_exp(2)
        rv[1] = emit_recip_vsc(1, se[1][1])
        emit_av(1, se[1][0], rv[1])
        emit_qk(3)
        for h in range(3, N_HEADS):
            se[h] = emit_scores_exp(h)
            rv[h - 1] = emit_recip_vsc(h - 1, se[h - 1][1])
            emit_av(h - 1, se[h - 1][0], rv[h - 1])
        rv[7] = emit_recip_vsc(7, se[7][1])
        emit_av(7, se[7][0], rv[7])

    # ---- output projection: Y = Wo^T.T @ O + bo ----
    for m in range(4):
        ps = sp.tile([128, 1024], F32, tag="sp", name=f"ps_y{m}")
        for n in range(2):
            for kc in range(4):
                _mm(nc, ps[:, n * 512:(n + 1) * 512],
                    WoT[:, kc, m * 128:(m + 1) * 128],
                    O[:, kc, n * 512:(n + 1) * 512],
                    kc == 0, kc == 3)
        nc.scalar.add(Y[:, m, :], ps[:], bo[:, m:m + 1])

    if not skip_out:
        nc.sync.dma_start(out=io["y"], in_=Y[:])


def build_nc(repeat: int = 1, variant: str = "full", compute_only_bench=False):
    nc = bacc.Bacc("TRN2", target_bir_lowering=False, debug=False)
    io = {
        "blob": nc.dram_tensor("blob", [128, BLOB_F], MM_DT,
                               kind="ExternalInput").ap(),
        "y": nc.dram_tensor("y", [128, 4, 1024], F32,
                            kind="ExternalOutput").ap(),
    }
    with tile.TileContext(nc) as tc:
        with ExitStack() as ctx:
            pools = make_pools(ctx, tc)
            if compute_only_bench:
                const = pools[0]
                blob = const.tile([128, BLOB_F], MM_DT, tag="blob")
                nc.sync.dma_start(out=blob[:], in_=io["blob"])
                for r in range(repeat):
                    _kernel_body(ctx, tc, io, pools, variant=variant,
                                 blob=blob, skip_out=(r < repeat - 1))
            else:
                for _ in range(repeat):
                    _kernel_body(ctx, tc, io, pools, variant=variant)
    nc.compile()
    return nc


def _pack_cmajor(a: np.ndarray, nchunk: int) -> np.ndarray:
    """[nchunk*128, F] -> [128, nchunk*F] with row r = chunk*128 + p."""
    f = a.shape[1]
    return a.reshape(nchunk, 128, f).transpose(1, 0, 2).reshape(128, nchunk * f)


def make_in_maps(x, context, Wq, bq, Wk, bk, Wv, bv, Wo, bo):
    shared = np.zeros((128, BLOB_F), np.float32)
    shared[:, OFF_WQ:OFF_WQ + 2048] = _pack_cmajor(
        np.ascontiguousarray(Wq.T) / 8.0, 4)
    shared[:, OFF_WK:OFF_WK + 2048] = _pack_cmajor(
        np.ascontiguousarray(Wk.T), 4)
    shared[:, OFF_WV:OFF_WV + 2048] = _pack_cmajor(
        np.ascontiguousarray(Wv.T), 4)
    shared[:, OFF_WO:OFF_WO + 2048] = _pack_cmajor(
        np.ascontiguousarray(Wo.T), 4)
    shared[:, OFF_BO:OFF_BO + 4] = bo.reshape(4, 128).T
    shared[0, OFF_BQR:OFF_BQR + 512] = bq / 8.0
    shared[0, OFF_BKR:OFF_BKR + 512] = bk
    shared[0, OFF_BVR:OFF_BVR + 512] = bv
    shared[0, OFF_ONES:OFF_ONES + 512] = 1.0

    in_maps = []
    for b in range(BS):
        blob = shared.copy()
        blob[:, OFF_X:OFF_X + 4096] = _pack_cmajor(x[b].reshape(C, HW), 4)
        blob[:, OFF_CTX:OFF_CTX + 4096] = _pack_cmajor(
            context[b].reshape(C, HW), 4)
        in_maps.append({"blob": blob})
    return in_maps


def kernel_with_results(inputs: dict, trace: bool = False, **run_kwargs):
    in_maps = make_in_maps(**{k: np.asarray(v, np.float32)
                              for k, v in inputs.items()})
    nc = build_nc()
    res = run_bass_kernel_spmd(nc, in_maps, core_ids=list(range(N_CORES)),
                               trace=trace, **run_kwargs)
    outs = []
    for r in res.results:
        y = r["y"]  # [128, 4, 1024]
        outs.append(y.transpose(1, 0, 2).reshape(C, H, W))
    return np.stack(outs).astype(np.float32), res


def kernel(**inputs) -> np.ndarray:
    out, _ = kernel_with_results(inputs)
    return out

